# revision 46
# baseline (speedup 1.0000x reference)
"""AttentionBlock (GroupNorm + 4-head attention with head_dim=128 + proj +
residual) on 8 Trainium2 NeuronCores, data-parallel over batch (2 per core).

Shapes (hardcoded): x [16, 512, 32, 32] f32; w_qkv [1536, 512]; w_proj [512, 512].
L = 1024, heads = 4 x 128, groupnorm 8 groups x 64 channels.

Layout / algorithm notes:
  - channels on partitions in 4 tiles of 128 (c = ct*128 + p)
  - GroupNorm stats: bn_stats per channel, cross-partition group reduce via a
    [128,2] mask matmul, broadcast back via a [2,128] mask matmul; rstd by
    Newton iteration on DVE.  xn written in place over x (residual is re-read
    from DRAM into the output tile later).
  - Q, K kept as [d=128, L] per head; V computed directly transposed (V^T) so
    attention needs no transposes: S^T = K^T Q (softmax axis on partitions),
    column sums via all-ones [128,128] matmul (simultaneously broadcast),
    1/s via a single approximate-reciprocal DVE op, AV contracts over
    partitions.
  - all big matmuls in float32r: full PE rate at ~2e-4 worst-case rel err.
  - batch phases are software-pipelined: batch1 GroupNorm runs early, batch1
    QKV matmul groups are interleaved into batch0's attention as PE filler
    (the attention inner loop is otherwise exp(ACT)-latency bound).
"""

import numpy as np

import concourse.bass as bass  # noqa: F401
import concourse.mybir as mybir
import concourse.tile as tile
from concourse import bacc
from concourse.bass_utils import run_bass_kernel_spmd
from concourse._compat import axon_active

AF = mybir.ActivationFunctionType
ALU = mybir.AluOpType
F32 = mybir.dt.float32
F32R = mybir.dt.float32r

N_CORES = 8
B = 16
C = 512
L = 1024
NH = 4
D = 128
G = 8
GS = C // G
P = 128
CT = C // P
BPC = B // N_CORES
EPS = 1e-5
SCALE = D ** -0.5
LH = 512


def build_kernel(loop_n=None, loop_stagger=False):
    """loop_n: if set, wrap the whole per-call body in an on-device For_i loop
    (used only for benchmarking true HW exec time per iteration)."""
    nc = bacc.Bacc(
        "TRN2", target_bir_lowering=False, debug=not axon_active(),
        num_devices=N_CORES,
    )

    x_d = nc.dram_tensor("x", [BPC, C, L], F32, kind="ExternalInput")
    gamma_d = nc.dram_tensor("gamma", [C], F32, kind="ExternalInput")
    beta_d = nc.dram_tensor("beta", [C], F32, kind="ExternalInput")
    wqkv_d = nc.dram_tensor("w_qkvT", [C, 3 * C], F32, kind="ExternalInput")
    bqkv_d = nc.dram_tensor("b_qkv", [3 * C], F32, kind="ExternalInput")
    wproj_d = nc.dram_tensor("w_projT", [C, C], F32, kind="ExternalInput")
    bproj_d = nc.dram_tensor("b_proj", [C], F32, kind="ExternalInput")
    mask01_d = nc.dram_tensor("mask01", [P, 2], F32, kind="ExternalInput")
    mask2_d = nc.dram_tensor("mask2", [2, P], F32, kind="ExternalInput")
    ones_d = nc.dram_tensor("ones", [P, P], F32, kind="ExternalInput")
    out_d = nc.dram_tensor("out", [BPC, C, L], F32, kind="ExternalOutput")

    with tile.TileContext(nc) as tc:
        with (
            tc.tile_pool(name="consts", bufs=1) as consts,
            tc.tile_pool(name="xq", bufs=2) as xq,        # x -> xn in place
            tc.tile_pool(name="qk", bufs=5) as qkp,       # per-head q / k
            tc.tile_pool(name="vp", bufs=2) as vp,
            tc.tile_pool(name="ep", bufs=3) as ep,
            tc.tile_pool(name="op", bufs=5) as op_,       # per-head attention out
            tc.tile_pool(name="rp", bufs=1) as rp,
            tc.tile_pool(name="outp", bufs=2) as outp,
            tc.tile_pool(name="sp", bufs=4) as sp,
            tc.tile_pool(name="ps_st", bufs=3, space="PSUM") as ps_st,
            tc.tile_pool(name="ps_fill", bufs=3, space="PSUM") as ps_fill,
            tc.tile_pool(name="ps_sums", bufs=1, space="PSUM") as ps_sums,
            tc.tile_pool(name="ps_av", bufs=1, space="PSUM") as ps_av,
        ):
            # ---------- constants ----------
            x0 = None
            x1 = None
            if not loop_n:
                # x of batch 0 first so GroupNorm starts ASAP; weights follow.
                x0 = xq.tile([P, CT, L], F32R, tag="x")
                for ct in range(CT):
                    nc.sync.dma_start(out=x0[:, ct, :], in_=x_d.ap().bitcast(F32R)[0, ct * P : (ct + 1) * P, :])

            # small constants first — the GroupNorm critical path needs the
            # masks and gamma/beta long before the big weight matrices.
            mask01 = consts.tile([P, 2], F32)
            nc.sync.dma_start(out=mask01, in_=mask01_d.ap())
            mask2 = consts.tile([2, P], F32)
            nc.sync.dma_start(out=mask2, in_=mask2_d.ap())
            gamma_s = consts.tile([P, CT], F32)
            beta_s = consts.tile([P, CT], F32)
            bproj_s = consts.tile([P, CT], F32)
            for ct in range(CT):
                cs = slice(ct * P, (ct + 1) * P)
                nc.sync.dma_start(out=gamma_s[:, ct : ct + 1], in_=gamma_d.ap()[cs, None])
                nc.sync.dma_start(out=beta_s[:, ct : ct + 1], in_=beta_d.ap()[cs, None])
                nc.sync.dma_start(out=bproj_s[:, ct : ct + 1], in_=bproj_d.ap()[cs, None])
            bqkv_s = consts.tile([P, 12], F32)
            for ot in range(12):
                nc.sync.dma_start(out=bqkv_s[:, ot : ot + 1],
                                  in_=bqkv_d.ap()[ot * P : (ot + 1) * P, None])
            ones_s = consts.tile([P, P], F32R)
            nc.sync.dma_start(out=ones_s, in_=ones_d.ap().bitcast(F32R))
            # w_qkv split by destination (q cols, k cols, v cols) so the
            # first qkv matmul groups unblock before the whole 3MB arrives.
            wqkv_s = consts.tile([P, CT, 3 * C], F32R)
            wproj_s = consts.tile([P, CT, C], F32R)
            if not loop_n:
                x1 = xq.tile([P, CT, L], F32R, tag="x")
                for oc in range(3):
                    ocs = slice(oc * C, (oc + 1) * C)
                    if oc == 2:
                        for ct in range(CT):
                            nc.sync.dma_start(out=x1[:, ct, :],
                                              in_=x_d.ap().bitcast(F32R)[1, ct * P : (ct + 1) * P, :])
                    for ct in range(CT):
                        cs = slice(ct * P, (ct + 1) * P)
                        nc.sync.dma_start(out=wqkv_s[:, ct, ocs],
                                          in_=wqkv_d.ap().bitcast(F32R)[cs, ocs])
            else:
                for ct in range(CT):
                    cs = slice(ct * P, (ct + 1) * P)
                    nc.sync.dma_start(out=wqkv_s[:, ct, :],
                                      in_=wqkv_d.ap().bitcast(F32R)[cs, :])

            # ---------- phase builders ----------
            def load_x(b):
                x_s = xq.tile([P, CT, L], F32R, tag="x")
                for ct in range(CT):
                    nc.sync.dma_start(out=x_s[:, ct, :],
                                      in_=x_d.ap().bitcast(F32R)[b, ct * P : (ct + 1) * P, :])
                return x_s

            def groupnorm(x_s):
                """Normalize x_s in place (tile is f32r; stats read it as f32)."""
                xf = x_s.bitcast(F32)
                s_stat = sp.tile([P, 8], F32, tag="s_stat")
                mv_all = sp.tile([P, CT, 2], F32, tag="mv_all")
                for ct in range(CT):
                    st6 = sp.tile([P, 2, 6], F32, tag="st6")
                    nc.vector.bn_stats(out=st6[:, 0, :], in_=xf[:, ct, 0:512])
                    nc.vector.bn_stats(out=st6[:, 1, :], in_=xf[:, ct, 512:1024])
                    nc.vector.bn_aggr(out=mv_all[:, ct, :], in_=st6)
                nc.vector.tensor_copy(out=s_stat[:, 0:4], in_=mv_all[:, :, 0])
                nc.vector.tensor_tensor(out=s_stat[:, 4:8], in0=mv_all[:, :, 0],
                                        in1=mv_all[:, :, 0], op=ALU.mult)
                nc.vector.tensor_tensor(out=s_stat[:, 4:8], in0=s_stat[:, 4:8],
                                        in1=mv_all[:, :, 1], op=ALU.add)
                gstat = ps_av.tile([2, 8], F32, tag="av")
                nc.tensor.matmul(gstat, lhsT=mask01, rhs=s_stat, start=True, stop=True)
                mean_g = sp.tile([2, 4], F32, tag="mean_g")
                nc.vector.tensor_scalar_mul(mean_g, gstat[:, 0:4], 1.0 / GS)
                var_g = sp.tile([2, 4], F32, tag="var_g")
                nc.vector.tensor_scalar_mul(var_g, gstat[:, 4:8], 1.0 / GS)
                msq = sp.tile([2, 4], F32, tag="msq")
                nc.vector.tensor_tensor(out=msq, in0=mean_g, in1=mean_g, op=ALU.mult)
                nc.vector.tensor_tensor(out=var_g, in0=var_g, in1=msq, op=ALU.subtract)
                # rstd = 1/sqrt(var+eps): Newton on DVE, seed min(1, 1/a)
                bsrc = sp.tile([2, 8], F32, tag="bsrc")
                a_t = sp.tile([2, 4], F32, tag="a_t")
                nc.vector.tensor_scalar_add(a_t, var_g, EPS)
                y_t = sp.tile([2, 4], F32, tag="y_t")
                nc.vector.reciprocal(out=y_t, in_=a_t)
                nc.vector.tensor_scalar(out=y_t, in0=y_t, scalar1=1.0, scalar2=1.0,
                                        op0=ALU.min, op1=ALU.mult)
                hy = sp.tile([2, 4], F32, tag="hy")
                t_t = sp.tile([2, 4], F32, tag="t_t")
                for it in range(4):
                    nc.vector.tensor_tensor(out=hy, in0=y_t, in1=y_t, op=ALU.mult)
                    nc.vector.tensor_tensor(out=t_t, in0=a_t, in1=hy, op=ALU.mult)
                    nc.vector.tensor_scalar(out=t_t, in0=t_t, scalar1=-0.5, scalar2=1.5,
                                            op0=ALU.mult, op1=ALU.add)
                    dst = bsrc[:, 4:8] if it == 3 else y_t
                    nc.vector.tensor_tensor(out=dst, in0=y_t, in1=t_t, op=ALU.mult)
                # bsrc[:,0:4] = +mean*rstd (sign handled at betap)
                nc.vector.tensor_tensor(out=bsrc[:, 0:4], in0=mean_g, in1=bsrc[:, 4:8], op=ALU.mult)
                bc = ps_sums.tile([P, 8], F32, tag="sums")
                nc.tensor.matmul(bc, lhsT=mask2, rhs=bsrc, start=True, stop=True)
                alpha = sp.tile([P, CT], F32, tag="alpha")
                nc.vector.tensor_tensor(out=alpha, in0=gamma_s, in1=bc[:, 4:8], op=ALU.mult)
                betap = sp.tile([P, CT], F32, tag="betap")
                nc.vector.tensor_tensor(out=betap, in0=gamma_s, in1=bc[:, 0:4], op=ALU.mult)
                nc.vector.tensor_tensor(out=betap, in0=beta_s, in1=betap, op=ALU.subtract)
                for ct in range(CT):
                    nc.vector.tensor_scalar(
                        out=x_s[:, ct, :], in0=xf[:, ct, :],
                        scalar1=alpha[:, ct : ct + 1], scalar2=betap[:, ct : ct + 1],
                        op0=ALU.mult, op1=ALU.add,
                    )

            def qkv_groups(x_s, q_t, k_t, vT_s, evict="act"):
                """Return a list of closures, each emitting one PE matmul group
                (4 accumulating matmuls into one psum tile) + its evict.
                evict: "act" when ACT is otherwise idle (batch-0 QKV burst),
                "dve" when the groups run as filler inside attention (ACT is
                busy with softmax exp there)."""
                xr = x_s
                groups = []

                def qk_group(ot, lc):
                    def emit():
                        mm = ps_fill.tile([P, LH], F32, tag="fill")
                        for ct in range(CT):
                            nc.tensor.matmul(
                                mm,
                                lhsT=wqkv_s[:, ct, ot * P : (ot + 1) * P],
                                rhs=xr[:, ct, lc * LH : (lc + 1) * LH],
                                start=(ct == 0), stop=(ct == CT - 1),
                            )
                        dst = (q_t if ot < 4 else k_t)[ot % 4][:, lc * LH : (lc + 1) * LH]
                        if evict == "act":
                            nc.scalar.add(out=dst, in_=mm, add=bqkv_s[:, ot : ot + 1])
                        else:
                            nc.vector.tensor_scalar_add(dst, mm, bqkv_s[:, ot : ot + 1])
                    return emit

                def v_group(lc):
                    def emit():
                        mm = ps_fill.tile([P, LH], F32, tag="fill")
                        for ct in range(CT):
                            nc.tensor.matmul(
                                mm,
                                lhsT=xr[:, ct, lc * P : (lc + 1) * P],
                                rhs=wqkv_s[:, ct, 2 * C : 3 * C],
                                start=(ct == 0), stop=(ct == CT - 1),
                            )
                        nc.vector.tensor_copy(out=vT_s[:, lc, :], in_=mm)
                    return emit

                for ot in range(8):
                    for lc in range(2):
                        groups.append(qk_group(ot, lc))
                for lc in range(8):
                    groups.append(v_group(lc))
                return groups

            def attn_head(h, q_h, k_h, vT_s, fillers, pop_every=2):
                """One head of attention, processed in two L-halves so each
                PSUM tile is a single bank; pops PE-filler closures between
                m-chunks to cover the exp(ACT) latency."""
                o_h = op_.tile([P, L], F32R, tag="o")
                for lh in range(2):
                    sl = slice(lh * LH, (lh + 1) * LH)
                    sums = ps_sums.tile([P, LH], F32, tag="sums")
                    av = ps_av.tile([P, LH], F32, tag="av")
                    for mc in range(8):
                        st = ps_st.tile([P, LH], F32, tag="st")
                        nc.tensor.matmul(
                            st,
                            lhsT=k_h[:, mc * P : (mc + 1) * P],
                            rhs=q_h[:, sl],
                            start=True, stop=True,
                        )
                        ex = ep.tile([P, LH], F32R)
                        nc.scalar.activation(out=ex, in_=st, func=AF.Exp, scale=SCALE)
                        nc.tensor.matmul(
                            sums, lhsT=ones_s, rhs=ex,
                            start=(mc == 0), stop=(mc == 7),
                        )
                        nc.tensor.matmul(
                            av,
                            lhsT=vT_s[:, mc, h * P : (h + 1) * P],
                            rhs=ex,
                            start=(mc == 0), stop=(mc == 7),
                        )
                        if mc % pop_every == pop_every - 1 and fillers:
                            fillers.pop(0)()
                    recip = rp.tile([P, LH], F32, tag="recip")
                    nc.vector.reciprocal_approx_fast(out=recip, in_=sums)
                    nc.vector.tensor_tensor(out=o_h[:, sl], in0=av, in1=recip, op=ALU.mult)
                return o_h

            def proj_groups(b, o_t, out_s):
                """Residual is preloaded into out_s by DMA; evict adds psum+bias;
                each finished row-block is DMA'd out immediately."""
                groups = []

                def pre():
                    for ct in range(CT):
                        nc.sync.dma_start(out=out_s[:, ct, :],
                                          in_=x_d.ap()[b, ct * P : (ct + 1) * P, :])

                def group(ot, lc):
                    def emit():
                        sl = slice(lc * LH, (lc + 1) * LH)
                        mm = ps_fill.tile([P, LH], F32, tag="fill")
                        for ct in range(CT):
                            nc.tensor.matmul(
                                mm,
                                lhsT=wproj_s[:, ct, ot * P : (ot + 1) * P],
                                rhs=o_t[ct][:, sl],
                                start=(ct == 0), stop=(ct == CT - 1),
                            )
                        nc.vector.scalar_tensor_tensor(
                            out=out_s[:, ot, sl], in0=mm,
                            scalar=bproj_s[:, ot : ot + 1], in1=out_s[:, ot, sl],
                            op0=ALU.add, op1=ALU.add,
                        )
                        if lc == 1:
                            nc.sync.dma_start(
                                out=out_d.ap()[b, ot * P : (ot + 1) * P, :],
                                in_=out_s[:, ot, :])
                    return emit

                pre()
                for ot in range(CT):
                    for lc in range(2):
                        groups.append(group(ot, lc))
                return groups

            # ---------- schedule ----------
            def schedule(x0, x1):
                # A0: groupnorm batch 0
                groupnorm(x0)
                q0 = [qkp.tile([P, L], F32R, tag="q", name=f"q0_{i}") for i in range(NH)]
                k0 = [qkp.tile([P, L], F32R, tag="k", name=f"k0_{i}") for i in range(NH)]
                vT0 = vp.tile([P, 8, C], F32R, tag="v")
                # B0: batch-0 qkv (dense PE burst)
                for g in qkv_groups(x0, q0, k0, vT0):
                    g()
                # w_proj is not needed until proj0 — load late
                for ct in range(CT):
                    cs = slice(ct * P, (ct + 1) * P)
                    nc.sync.dma_start(out=wproj_s[:, ct, :],
                                      in_=wproj_d.ap().bitcast(F32R)[cs, :])
                groupnorm(x1)
                q1 = [qkp.tile([P, L], F32R, tag="q", name=f"q1_{i}") for i in range(NH)]
                k1 = [qkp.tile([P, L], F32R, tag="k", name=f"k1_{i}") for i in range(NH)]
                vT1 = vp.tile([P, 8, C], F32R, tag="v")
                b1_fill = qkv_groups(x1, q1, k1, vT1)
                # C0: batch-0 attention with batch-1 qkv as PE filler
                o0 = []
                for h in range(NH):
                    o0.append(attn_head(h, q0[h], k0[h], vT0, b1_fill, pop_every=3))
                for g in b1_fill:
                    g()
                # D0: batch-0 proj; C1: batch-1 attention with proj0 as filler
                out0 = outp.tile([P, CT, L], F32, tag="out")
                d0_fill = proj_groups(0, o0, out0)
                o1 = []
                for h in range(NH):
                    o1.append(attn_head(h, q1[h], k1[h], vT1, d0_fill, pop_every=4))
                for g in d0_fill:
                    g()
                # D1: batch-1 proj + store
                out1 = outp.tile([P, CT, L], F32, tag="out")
                for g in proj_groups(1, o1, out1):
                    g()

            if loop_n:
                with tc.For_i(0, loop_n, 1, staggered_reset=loop_stagger):
                    x0i = load_x(0)
                    x1i = load_x(1)
                    schedule(x0i, x1i)
            else:
                schedule(x0, x1)

    nc.compile()
    return nc


_NC_CACHE = None


def _get_nc():
    global _NC_CACHE
    if _NC_CACHE is None:
        _NC_CACHE = build_kernel()
    return _NC_CACHE


def kernel(x, gamma, beta, w_qkv, b_qkv, w_proj, b_proj, **_ignored):
    x = np.asarray(x, dtype=np.float32)
    b, c, h, w = x.shape
    assert (b, c, h * w) == (B, C, L)
    xf = np.ascontiguousarray(x.reshape(B, C, L))
    wqkvT = np.ascontiguousarray(np.asarray(w_qkv, np.float32).T)
    wprojT = np.ascontiguousarray(np.asarray(w_proj, np.float32).T)
    # v-bias passes through the attention average unchanged; fold through proj
    b_v = np.asarray(b_qkv, np.float64)[2 * C :]
    b_proj_eff = (np.asarray(b_proj, np.float64)
                  + np.asarray(w_proj, np.float64) @ b_v).astype(np.float32)
    mask01 = np.zeros((P, 2), np.float32)
    mask01[:GS, 0] = 1.0
    mask01[GS:, 1] = 1.0
    common = {
        "gamma": np.ascontiguousarray(np.asarray(gamma, np.float32)),
        "beta": np.ascontiguousarray(np.asarray(beta, np.float32)),
        "w_qkvT": wqkvT,
        "b_qkv": np.ascontiguousarray(np.asarray(b_qkv, np.float32)),
        "w_projT": wprojT,
        "b_proj": np.ascontiguousarray(b_proj_eff),
        "mask01": mask01,
        "mask2": np.ascontiguousarray(mask01.T),
        "ones": np.ones((P, P), np.float32),
    }
    in_maps = [
        {"x": np.ascontiguousarray(xf[i * BPC : (i + 1) * BPC]), **common}
        for i in range(N_CORES)
    ]
    nc = _get_nc()
    res = run_bass_kernel_spmd(nc, in_maps, core_ids=list(range(N_CORES)))
    out = np.concatenate([res.results[i]["out"] for i in range(N_CORES)], axis=0)
    return out.reshape(B, C, h, w).astype(np.float32)
